# revision 1
# baseline (speedup 1.0000x reference)
"""DCNv3 forward on 8 trn2 NeuronCores.

Strategy (data-parallel over (batch, H-half) -> 8 shards):
  - host: pre-transpose per-shard input into the layouts the device wants
    (zero-padded pixel slab for sampling; CHW tile for the offset/mask matmuls)
  - device per core, per 4-row tile:
      PE matmul  : offsets (72) + mask logits (36) for 512 pixels
      PE transp  : move results to pixel-on-partition layout
      ACT        : exp, hat weights  relu(1 - |u - i|)
      DVE        : softmax-normalize, per-cell coefficients m*hy_i*hx_j
      DVE apply  : acc[g-slice] (+)= coef[(g,p,i,j)][wo] * Xshift[wo, c-slice]
                   (fused scalar_tensor_tensor, per-partition scalar)
  - bilinear gather is made gather-free: integer part of every sampling
    offset is bounded (|off|<=~2px), so sampling = sum over a per-(g,p)
    3-4 tap window of hat-weighted *fixed* shifts of the input, and every
    fixed shift is just an access-pattern offset into an SBUF slab.
"""

import numpy as np
import sys

sys.path.insert(0, "/opt/trn_rl_repo")

import concourse.bass as bass
import concourse.bacc as bacc
import concourse.mybir as mybir
import concourse.tile as tile
from concourse.bass_utils import run_bass_kernel_spmd

B, C, H, W = 4, 128, 128, 128
G, P, gc = 4, 9, 32
N_CORES = 8
HS = H // 2          # rows per core (b, half)
RT = 4               # output rows per device tile
NTILES = HS // RT    # 16
NTAP = 4             # hat taps per axis (span is 3 or 4 per (g,p))

f32 = mybir.dt.float32

_KS = np.array([-1.0, 0.0, 1.0], np.float32)
KX = np.repeat(_KS, 3)   # x-major flatten (matches torch meshgrid in ref)
KY = np.tile(_KS, 3)


def _geometry(inp, W_off, b_off):
    """Per-(g,p) integer tap bases/spans from the actual offset field."""
    xhw = inp.reshape(B, H, W, C)
    off = (xhw.reshape(-1, C) @ W_off + b_off).reshape(-1, G, P, 2)
    rx = off[..., 0] + KX          # offset (x) relative to wo+1  (padded coords)
    ry = off[..., 1] + KY
    Bx = np.floor(rx.min(axis=0)).astype(np.int64)
    By = np.floor(ry.min(axis=0)).astype(np.int64)
    spx = np.floor(rx.max(axis=0)).astype(np.int64) + 2 - Bx
    spy = np.floor(ry.max(axis=0)).astype(np.int64) + 2 - By
    spx = np.minimum(spx, NTAP)
    spy = np.minimum(spy, NTAP)
    assert spx.max() <= NTAP and spy.max() <= NTAP
    return Bx, By, spx, spy


class _Geom:
    pass


def _build(g: "_Geom"):
    nc = bacc.Bacc("TRN2", target_bir_lowering=False, debug=False,
                   num_devices=N_CORES)

    xslab_t = nc.dram_tensor("xslab", [g.NROW * g.NCOL * C], f32, kind="ExternalInput")
    xchw_t = nc.dram_tensor("xchw", [C, HS * W], f32, kind="ExternalInput")
    wcat_t = nc.dram_tensor("wcat", [C, 108], f32, kind="ExternalInput")
    addc_t = nc.dram_tensor("addc", [C, 108], f32, kind="ExternalInput")
    ident_t = nc.dram_tensor("ident", [C, C], f32, kind="ExternalInput")
    cvals_t = nc.dram_tensor("cvals", [C, 8], f32, kind="ExternalInput")
    out_t = nc.dram_tensor("out", [HS * W * C], f32, kind="ExternalOutput")

    NS, NR = g.NS, g.NR
    mult, add = mybir.AluOpType.mult, mybir.AluOpType.add
    AF = mybir.ActivationFunctionType

    def vap(v, off, dims):
        return bass.AP(tensor=v.tensor, offset=v.offset + off, ap=[v.ap[0]] + dims)

    with tile.TileContext(nc) as tc:
        with (
            tc.tile_pool(name="const", bufs=1) as cpool,
            tc.tile_pool(name="xs", bufs=2) as xspool,
            tc.tile_pool(name="work", bufs=2) as wpool,
            tc.tile_pool(name="psum", bufs=2, space="PSUM") as pspool,
        ):
            wcat0 = cpool.tile([C, 108], f32)
            wcat = cpool.tile([C, 108], f32)
            addc = cpool.tile([C, 108], f32)
            ident = cpool.tile([C, C], f32)
            cvals = cpool.tile([C, 8], f32)
            nc.sync.dma_start(wcat0[:], wcat_t.ap())
            nc.sync.dma_start(addc[:], addc_t.ap())
            nc.sync.dma_start(ident[:], ident_t.ap())
            nc.sync.dma_start(cvals[:], cvals_t.ap())
            # matmul operands come via ACT copies: the Matmult HW struct has a
            # single sync-wait slot, so all its deps must arrive on one sem
            nc.scalar.copy(wcat[:], wcat0[:])

            for t in range(NTILES):
                # ---- loads -------------------------------------------------
                xs = xspool.tile([C, NS * NR * C], f32, name="xs")
                for si in range(NS):
                    src = bass.AP(
                        tensor=xslab_t,
                        offset=(RT * t * g.NCOL + g.C0 + si) * C,
                        ap=[[C, W], [g.NCOL * C, NR], [1, C]])
                    nc.sync.dma_start(
                        vap(xs[:], si * NR * C, [[C, NR], [1, C]]), src)

                xc0 = wpool.tile([C, RT * W], f32, name="xc0")
                nc.sync.dma_start(
                    xc0[:], bass.AP(tensor=xchw_t, offset=RT * t * W,
                                    ap=[[HS * W, C], [1, RT * W]]))
                xc = wpool.tile([C, RT * W], f32, name="xc")
                nc.scalar.copy(xc[:], xc0[:])

                # ---- offsets / logits (PE), directly in q-on-partitions ----
                rawq = wpool.tile([C, RT * 108], f32, name="rawq")
                for k in range(RT):
                    praw = pspool.tile([C, 108], f32, name="praw")
                    nc.tensor.matmul(praw[:], xc[:, k * W:(k + 1) * W],
                                     wcat[:], start=True, stop=True)
                    nc.scalar.copy(vap(rawq[:], k * 108, [[1, 108]]), praw[:])

                # rawq[:, r*108 + k] : k 0..35 y-offs, 36..71 x-offs, 72..107 logits
                nc.vector.tensor_tensor(
                    vap(rawq[:], 0, [[108, RT], [1, 108]]),
                    vap(rawq[:], 0, [[108, RT], [1, 108]]),
                    vap(addc[:], 0, [[0, RT], [1, 108]]), add)

                # ---- softmax (unnormalized exp -> normalize) ---------------
                el = wpool.tile([C, RT * 36], f32, name="el")
                nc.scalar.activation(
                    vap(el[:], 0, [[36, RT], [1, 36]]),
                    vap(rawq[:], 72, [[108, RT], [1, 36]]), AF.Exp)
                den = wpool.tile([C, RT * G], f32, name="den")
                nc.vector.tensor_reduce(
                    vap(den[:], 0, [[G, RT], [1, G]]),
                    vap(el[:], 0, [[36, RT], [9, G], [1, P]]),
                    mybir.AxisListType.X, add)
                denr = wpool.tile([C, RT * G], f32, name="denr")
                nc.vector.reciprocal(denr[:], den[:])
                nc.vector.tensor_tensor(
                    vap(el[:], 0, [[36, RT], [9, G], [1, P]]),
                    vap(el[:], 0, [[36, RT], [9, G], [1, P]]),
                    vap(denr[:], 0, [[G, RT], [1, G], [0, P]]), mult)

                # ---- hat weights ------------------------------------------
                hats = []
                for i in range(NTAP):
                    habs = wpool.tile([C, RT * 72], f32, name=f"habs{i}")
                    nc.scalar.activation(
                        vap(habs[:], 0, [[72, RT], [1, 72]]),
                        vap(rawq[:], 0, [[108, RT], [1, 72]]),
                        AF.Abs, bias=cvals[:, i:i + 1])
                    h = wpool.tile([C, RT * 72], f32, name=f"hat{i}")
                    nc.scalar.activation(h[:], habs[:], AF.Relu,
                                         bias=cvals[:, 4:5], scale=-1.0)
                    hats.append(h)

                # ---- per-cell coefficients --------------------------------
                mh = []
                for i in range(NTAP):
                    mt = wpool.tile([C, RT * 36], f32, name=f"mh{i}")
                    nc.vector.tensor_tensor(
                        vap(mt[:], 0, [[36, RT], [1, 36]]),
                        vap(el[:], 0, [[36, RT], [1, 36]]),
                        vap(hats[i][:], 0, [[72, RT], [1, 36]]), mult)
                    mh.append(mt)
                coef = wpool.tile([C, NTAP * NTAP * RT * 36], f32, name="coef")
                for i in range(NTAP):
                    for j in range(NTAP):
                        s = i * NTAP + j
                        nc.vector.tensor_tensor(
                            vap(coef[:], s * RT * 36, [[36, RT], [1, 36]]),
                            vap(mh[i][:], 0, [[36, RT], [1, 36]]),
                            vap(hats[j][:], 36, [[72, RT], [1, 36]]), mult)

                # ---- apply ------------------------------------------------
                acc = wpool.tile([C, RT * C], f32, name="acc")
                for r in range(RT):
                    for gg in range(G):
                        first = True
                        aslice = vap(acc[:], r * C + gg * gc, [[1, gc]])
                        for p in range(P):
                            by, bx = int(g.By[gg, p]), int(g.Bx[gg, p])
                            for i in range(int(g.spy[gg, p])):
                                rho = r + 2 + by + i
                                for j in range(int(g.spx[gg, p])):
                                    si = bx + j - g.SMIN
                                    xv = vap(xs[:], (si * NR + rho) * C + gg * gc, [[1, gc]])
                                    cidx = (i * NTAP + j) * RT * 36 + r * 36 + gg * P + p
                                    ccol = vap(coef[:], cidx, [[1, 1]])
                                    if first:
                                        nc.vector.tensor_scalar_mul(aslice, xv, ccol)
                                        first = False
                                    else:
                                        nc.vector.scalar_tensor_tensor(
                                            aslice, xv, ccol, aslice, mult, add)

                nc.sync.dma_start(
                    bass.AP(tensor=out_t, offset=RT * t * W * C,
                            ap=[[C, W], [W * C, RT], [1, C]]),
                    vap(acc[:], 0, [[C, RT], [1, C]]))

    nc.compile()
    return nc


def _host_prep(inp, W_off, b_off, W_mask, b_mask, g):
    xhw = inp.reshape(B, H, W, C)

    wcat = np.empty((C, 108), np.float32)
    addc_row = np.empty(108, np.float32)
    for gg in range(G):
        for p in range(P):
            gp = gg * P + p
            wcat[:, gp] = W_off[:, 2 * gp + 1]           # y
            wcat[:, 36 + gp] = W_off[:, 2 * gp]          # x
            wcat[:, 72 + gp] = W_mask[:, gp]
            addc_row[gp] = b_off[2 * gp + 1] + (KY[p] - g.By[gg, p])
            addc_row[36 + gp] = b_off[2 * gp] + (KX[p] - g.Bx[gg, p])
            addc_row[72 + gp] = b_mask[gp]
    addc = np.tile(addc_row[None, :], (C, 1))
    ident = np.eye(C, dtype=np.float32)
    cvals = np.zeros((C, 8), np.float32)
    for i in range(NTAP):
        cvals[:, i] = -float(i)
    cvals[:, 4] = 1.0

    in_maps = []
    for core in range(N_CORES):
        b, half = divmod(core, 2)
        h0 = HS * half
        # slab rows: padded rows [h0-1, h0-1+NROW) ; cols: padded [-2, NCOL-2)
        xslab = np.zeros((g.NROW, g.NCOL, C), np.float32)
        for lr in range(g.NROW):
            orig = lr + h0 - 2
            if 0 <= orig < H:
                xslab[lr, 3:3 + W, :] = xhw[b, orig]
        xchw = np.ascontiguousarray(
            xhw[b, h0:h0 + HS].reshape(HS * W, C).T)
        in_maps.append({
            "xslab": xslab.reshape(-1),
            "xchw": xchw,
            "wcat": wcat,
            "addc": addc,
            "ident": ident,
            "cvals": cvals,
        })
    return in_maps


def _make_geom(inp, W_off, b_off):
    g = _Geom()
    g.Bx, g.By, g.spx, g.spy = _geometry(inp, W_off, b_off)
    g.SMIN = int(g.Bx.min())
    smax = int((g.Bx + g.spx - 1).max())
    g.NS = smax - g.SMIN + 1
    rmin = int(2 + g.By.min())            # rho = r+2+By+i ; r=0,i=0
    rmax = int(RT - 1 + 2 + (g.By + g.spy - 1).max())
    assert rmin >= 0
    g.NR = rmax + 1
    g.NROW = RT * (NTILES - 1) + g.NR     # slab rows per core
    # slab col for (wo, si): wo + si + (3 + SMIN) ; worst col = 127+NS-1+3+SMIN
    g.C0 = 3 + g.SMIN                     # col offset baked into slab layout
    g.NCOL = W + g.NS - 1 + g.C0 + 1
    return g


def _run(inp, W_off, b_off, W_mask, b_mask, **spmd_kwargs):
    inp = np.ascontiguousarray(inp, np.float32)
    g = _make_geom(inp, np.asarray(W_off, np.float32), np.asarray(b_off, np.float32))
    nc = _build(g)
    in_maps = _host_prep(inp, np.asarray(W_off, np.float32),
                         np.asarray(b_off, np.float32),
                         np.asarray(W_mask, np.float32),
                         np.asarray(b_mask, np.float32), g)
    res = run_bass_kernel_spmd(nc, in_maps, core_ids=list(range(N_CORES)),
                               **spmd_kwargs)
    out = np.empty((B, H, W, C), np.float32)
    for core in range(N_CORES):
        b, half = divmod(core, 2)
        out[b, HS * half:HS * (half + 1)] = \
            res.results[core]["out"].reshape(HS, W, C)
    return out.reshape(B, C, H, W), res


def kernel(inp, W_off, b_off, W_mask, b_mask):
    out, _ = _run(inp, W_off, b_off, W_mask, b_mask)
    return out


if __name__ == "__main__":
    d = np.load("/root/problem/ref_cache.npz")
    got = kernel(d["inp"], d["W_off"], d["b_off"], d["W_mask"], d["b_mask"])
    exp = d["exp"]
    err = np.abs(got - exp).max()
    print("absmax err:", err, "rel:", err / np.abs(exp).max())



# revision 2
# speedup vs baseline: 3.0104x; 3.0104x over previous
"""DCNv3 forward on 8 trn2 NeuronCores.

Strategy (data-parallel over (batch, H-half) -> 8 shards):
  - host: pre-transpose per-shard input into the layouts the device wants
    (zero-padded pixel slab for sampling; CHW tile for the offset/mask matmuls)
  - device per core, per 4-row tile:
      PE matmul  : offsets (72) + mask logits (36) for 512 pixels
      ACT        : exp, hat weights relu(1 - |u - t|) for a COMMON absolute
                   tap window (UxV cells shared by every (g,p))
      DVE        : softmax-normalize; one big outer-product op building
                   per-cell coefficients A[q,g,u,v] = sum_p m_p*hy_p(u)*hx_p(v)
                   (one 5k-elem mult + one 5k-elem reduce over p);
                   apply = per used cell one 512-elem mult into a cell-strided
                   tmp, then ONE 16k-elem reduce over cells into acc.
  - bilinear gather is gather-free: integer parts of all sampling offsets are
    bounded, so sampling = hat-weighted fixed shifts of the input; every fixed
    shift is an access-pattern offset into an SBUF slab (V column-shifted
    copies of the row slab).
"""

import numpy as np
import sys

sys.path.insert(0, "/opt/trn_rl_repo")

import concourse.bass as bass
import concourse.bacc as bacc
import concourse.mybir as mybir
import concourse.tile as tile
from concourse.bass_utils import run_bass_kernel_spmd

B, C, H, W = 4, 128, 128, 128
G, P, gc = 4, 9, 32
N_CORES = 8
HS = H // 2          # rows per core (b, half)
RT = 4               # output rows per device tile
NTILES = HS // RT    # 16

f32 = mybir.dt.float32

_KS = np.array([-1.0, 0.0, 1.0], np.float32)
KX = np.repeat(_KS, 3)   # x-major flatten (matches torch meshgrid in ref)
KY = np.tile(_KS, 3)


def _geometry(inp, W_off, b_off):
    """Global tap window + used-cell mask from the actual offset field."""
    xhw = inp.reshape(B, H, W, C)
    off = (xhw.reshape(-1, C) @ W_off + b_off).reshape(-1, G, P, 2)
    rx = off[..., 0] + KX          # offset (x) relative to wo+1  (padded coords)
    ry = off[..., 1] + KY
    Bx = np.floor(rx.min(axis=0)).astype(np.int64)
    By = np.floor(ry.min(axis=0)).astype(np.int64)
    spx = np.floor(rx.max(axis=0)).astype(np.int64) + 2 - Bx
    spy = np.floor(ry.max(axis=0)).astype(np.int64) + 2 - By
    g = _Geom()
    g.DX0 = int(Bx.min())
    g.DY0 = int(By.min())
    g.V = int((Bx + spx).max()) - g.DX0
    g.U = int((By + spy).max()) - g.DY0
    used = np.zeros((g.U, g.V), bool)
    for gg in range(G):
        for p in range(P):
            u0 = By[gg, p] - g.DY0
            v0 = Bx[gg, p] - g.DX0
            used[u0:u0 + spy[gg, p], v0:v0 + spx[gg, p]] = True
    g.cells = [(u, v) for u in range(g.U) for v in range(g.V) if used[u, v]]
    return g


class _Geom:
    pass


def _build(g: "_Geom"):
    nc = bacc.Bacc("TRN2", target_bir_lowering=False, debug=False,
                   num_devices=N_CORES)

    U, V, NR = g.U, g.V, g.NR
    NCELL = len(g.cells)
    NTAP = max(U, V)

    xslab_t = nc.dram_tensor("xslab", [g.NROW * g.NCOL * C], f32, kind="ExternalInput")
    xchw_t = nc.dram_tensor("xchw", [C, HS * W], f32, kind="ExternalInput")
    wcat_t = nc.dram_tensor("wcat", [C, 108], f32, kind="ExternalInput")
    addc_t = nc.dram_tensor("addc", [C, 108], f32, kind="ExternalInput")
    cvals_t = nc.dram_tensor("cvals", [C, NTAP + 1], f32, kind="ExternalInput")
    out_t = nc.dram_tensor("out", [HS * W * C], f32, kind="ExternalOutput")

    mult, add = mybir.AluOpType.mult, mybir.AluOpType.add
    AF = mybir.ActivationFunctionType

    def vap(v, off, dims):
        return bass.AP(tensor=v.tensor, offset=v.offset + off, ap=[v.ap[0]] + dims)

    with tile.TileContext(nc) as tc:
        with (
            tc.tile_pool(name="const", bufs=1) as cpool,
            tc.tile_pool(name="xs", bufs=2) as xspool,
            tc.tile_pool(name="work", bufs=2) as wpool,
            tc.tile_pool(name="big", bufs=1) as bpool,
            tc.tile_pool(name="psum", bufs=2, space="PSUM") as pspool,
        ):
            wcat0 = cpool.tile([C, 108], f32)
            wcat = cpool.tile([C, 108], f32)
            addc = cpool.tile([C, 108], f32)
            cvals = cpool.tile([C, NTAP + 1], f32)
            nc.sync.dma_start(wcat0[:], wcat_t.ap())
            nc.sync.dma_start(addc[:], addc_t.ap())
            nc.sync.dma_start(cvals[:], cvals_t.ap())
            # matmul operands come via ACT copies: the Matmult HW struct has a
            # single sync-wait slot, so all its deps must arrive on one sem
            nc.scalar.copy(wcat[:], wcat0[:])

            for t in range(NTILES):
                # ---- loads -------------------------------------------------
                xs = xspool.tile([C, V * NR * C], f32, name="xs")
                for v in range(V):
                    src = bass.AP(
                        tensor=xslab_t,
                        offset=(RT * t * g.NCOL + g.C0 + v) * C,
                        ap=[[C, W], [g.NCOL * C, NR], [1, C]])
                    nc.sync.dma_start(
                        vap(xs[:], v * NR * C, [[C, NR], [1, C]]), src)

                xc0 = wpool.tile([C, RT * W], f32, name="xc0")
                nc.sync.dma_start(
                    xc0[:], bass.AP(tensor=xchw_t, offset=RT * t * W,
                                    ap=[[HS * W, C], [1, RT * W]]))
                xc = wpool.tile([C, RT * W], f32, name="xc")
                nc.scalar.copy(xc[:], xc0[:])

                # ---- offsets / logits (PE), directly in q-on-partitions ----
                rawq = wpool.tile([C, RT * 108], f32, name="rawq")
                for k in range(RT):
                    praw = pspool.tile([C, 108], f32, name="praw")
                    nc.tensor.matmul(praw[:], xc[:, k * W:(k + 1) * W],
                                     wcat[:], start=True, stop=True)
                    nc.scalar.copy(vap(rawq[:], k * 108, [[1, 108]]), praw[:])

                # rawq[:, r*108 + k] : k 0..35 y-offs, 36..71 x-offs, 72..107 logits
                nc.vector.tensor_tensor(
                    vap(rawq[:], 0, [[108, RT], [1, 108]]),
                    vap(rawq[:], 0, [[108, RT], [1, 108]]),
                    vap(addc[:], 0, [[0, RT], [1, 108]]), add)

                # ---- softmax (unnormalized exp -> normalize) ---------------
                el = wpool.tile([C, RT * 36], f32, name="el")
                nc.scalar.activation(
                    vap(el[:], 0, [[36, RT], [1, 36]]),
                    vap(rawq[:], 72, [[108, RT], [1, 36]]), AF.Exp)
                den = wpool.tile([C, RT * G], f32, name="den")
                nc.vector.tensor_reduce(
                    vap(den[:], 0, [[G, RT], [1, G]]),
                    vap(el[:], 0, [[36, RT], [9, G], [1, P]]),
                    mybir.AxisListType.X, add)
                denr = wpool.tile([C, RT * G], f32, name="denr")
                nc.vector.reciprocal(denr[:], den[:])
                nc.vector.tensor_tensor(
                    vap(el[:], 0, [[36, RT], [9, G], [1, P]]),
                    vap(el[:], 0, [[36, RT], [9, G], [1, P]]),
                    vap(denr[:], 0, [[G, RT], [1, G], [0, P]]), mult)

                # ---- hat weights at absolute taps 0..NTAP-1 (y and x) ------
                # hyx[wo, t*RT*72 + r*72 + (0..35 y | 36..71 x)]
                hyx = wpool.tile([C, NTAP * RT * 72], f32, name="hyx")
                habs = wpool.tile([C, NTAP * RT * 72], f32, name="habs")
                for tt in range(NTAP):
                    nc.scalar.activation(
                        vap(habs[:], tt * RT * 72, [[72, RT], [1, 72]]),
                        vap(rawq[:], 0, [[108, RT], [1, 72]]),
                        AF.Abs, bias=cvals[:, tt:tt + 1])
                    nc.scalar.activation(
                        vap(hyx[:], tt * RT * 72, [[72, RT], [1, 72]]),
                        vap(habs[:], tt * RT * 72, [[72, RT], [1, 72]]),
                        AF.Relu, bias=cvals[:, NTAP:NTAP + 1], scale=-1.0)

                # compact hx: hxc[wo, v*RT*36 + r*36 + gp]
                hxc = wpool.tile([C, V * RT * 36], f32, name="hxc")
                nc.scalar.copy(
                    vap(hxc[:], 0, [[RT * 36, V], [36, RT], [1, 36]]),
                    vap(hyx[:], 36, [[RT * 72, V], [72, RT], [1, 36]]))

                # ---- mh[u] = m * hy_u  (one op over (u, r, gp)) ------------
                mh = wpool.tile([C, U * RT * 36], f32, name="mh")
                nc.vector.tensor_tensor(
                    vap(mh[:], 0, [[RT * 36, U], [36, RT], [1, 36]]),
                    vap(el[:], 0, [[0, U], [36, RT], [1, 36]]),
                    vap(hyx[:], 0, [[RT * 72, U], [72, RT], [1, 36]]), mult)

                # ---- A[q,g,u,v] = sum_p mh_u,p * hx_v,p --------------------
                # prod[wo, (u*V+v)*144 + r*36 + gp]
                prod = bpool.tile([C, U * V * RT * 36], f32, name="prod")
                nc.vector.tensor_tensor(
                    vap(prod[:], 0, [[V * RT * 36, U], [RT * 36, V], [1, RT * 36]]),
                    vap(mh[:], 0, [[RT * 36, U], [0, V], [1, RT * 36]]),
                    vap(hxc[:], 0, [[0, U], [RT * 36, V], [1, RT * 36]]), mult)
                # A[wo, (u*V+v)*16 + r*4 + g]
                A = wpool.tile([C, U * V * RT * G], f32, name="A")
                nc.vector.tensor_reduce(
                    vap(A[:], 0, [[RT * G, U * V], [1, RT * G]]),
                    vap(prod[:], 0, [[RT * 36, U * V], [P, RT * G], [1, P]]),
                    mybir.AxisListType.X, add)

                # ---- apply: tmp[wo, rgc*NCELL + j] = xs_cell * A_cell ------
                tmp = bpool.tile([C, RT * C * NCELL], f32, name="tmp")
                for j, (u, v) in enumerate(g.cells):
                    rho0 = 2 + g.DY0 + u
                    nc.vector.tensor_tensor(
                        vap(tmp[:], j,
                            [[C * NCELL, RT], [gc * NCELL, G], [NCELL, gc]]),
                        vap(xs[:], (v * NR + rho0) * C,
                            [[C, RT], [gc, G], [1, gc]]),
                        vap(A[:], (u * V + v) * RT * G,
                            [[G, RT], [1, G], [0, gc]]), mult)

                acc = wpool.tile([C, RT * C], f32, name="acc")
                nc.vector.tensor_reduce(
                    vap(acc[:], 0, [[1, RT * C]]),
                    vap(tmp[:], 0, [[NCELL, RT * C], [1, NCELL]]),
                    mybir.AxisListType.X, add)

                nc.sync.dma_start(
                    bass.AP(tensor=out_t, offset=RT * t * W * C,
                            ap=[[C, W], [W * C, RT], [1, C]]),
                    vap(acc[:], 0, [[C, RT], [1, C]]))

    nc.compile()
    return nc


def _host_prep(inp, W_off, b_off, W_mask, b_mask, g):
    xhw = inp.reshape(B, H, W, C)
    NTAP = max(g.U, g.V)

    wcat = np.empty((C, 108), np.float32)
    addc_row = np.empty(108, np.float32)
    for gg in range(G):
        for p in range(P):
            gp = gg * P + p
            wcat[:, gp] = W_off[:, 2 * gp + 1]           # y
            wcat[:, 36 + gp] = W_off[:, 2 * gp]          # x
            wcat[:, 72 + gp] = W_mask[:, gp]
            addc_row[gp] = b_off[2 * gp + 1] + (KY[p] - g.DY0)
            addc_row[36 + gp] = b_off[2 * gp] + (KX[p] - g.DX0)
            addc_row[72 + gp] = b_mask[gp]
    addc = np.tile(addc_row[None, :], (C, 1))
    cvals = np.zeros((C, NTAP + 1), np.float32)
    for i in range(NTAP):
        cvals[:, i] = -float(i)
    cvals[:, NTAP] = 1.0

    in_maps = []
    for core in range(N_CORES):
        b, half = divmod(core, 2)
        h0 = HS * half
        # slab rows: padded rows [h0-1, h0-1+NROW) ; cols: padded [-2, NCOL-2)
        xslab = np.zeros((g.NROW, g.NCOL, C), np.float32)
        for lr in range(g.NROW):
            orig = lr + h0 - 2
            if 0 <= orig < H:
                xslab[lr, 3:3 + W, :] = xhw[b, orig]
        xchw = np.ascontiguousarray(
            xhw[b, h0:h0 + HS].reshape(HS * W, C).T)
        in_maps.append({
            "xslab": xslab.reshape(-1),
            "xchw": xchw,
            "wcat": wcat,
            "addc": addc,
            "cvals": cvals,
        })
    return in_maps


def _make_geom(inp, W_off, b_off):
    g = _geometry(inp, W_off, b_off)
    # slab row for output row r (in tile), tap u: rho = r + 2 + DY0 + u
    rmin = 2 + g.DY0
    assert rmin >= 0
    g.NR = RT - 1 + 2 + g.DY0 + g.U - 1 + 1   # max rho + 1
    g.NROW = RT * (NTILES - 1) + g.NR     # slab rows per core
    # slab col for (wo, v): wo + v + (3 + DX0)
    g.C0 = 3 + g.DX0                      # col offset baked into slab layout
    assert g.C0 >= 0
    g.NCOL = W + g.V - 1 + g.C0 + 1
    return g


def _run(inp, W_off, b_off, W_mask, b_mask, **spmd_kwargs):
    inp = np.ascontiguousarray(inp, np.float32)
    g = _make_geom(inp, np.asarray(W_off, np.float32), np.asarray(b_off, np.float32))
    nc = _build(g)
    in_maps = _host_prep(inp, np.asarray(W_off, np.float32),
                         np.asarray(b_off, np.float32),
                         np.asarray(W_mask, np.float32),
                         np.asarray(b_mask, np.float32), g)
    res = run_bass_kernel_spmd(nc, in_maps, core_ids=list(range(N_CORES)),
                               **spmd_kwargs)
    out = np.empty((B, H, W, C), np.float32)
    for core in range(N_CORES):
        b, half = divmod(core, 2)
        out[b, HS * half:HS * (half + 1)] = \
            res.results[core]["out"].reshape(HS, W, C)
    return out.reshape(B, C, H, W), res


def kernel(inp, W_off, b_off, W_mask, b_mask):
    out, _ = _run(inp, W_off, b_off, W_mask, b_mask)
    return out


if __name__ == "__main__":
    d = np.load("/root/problem/ref_cache.npz")
    got = kernel(d["inp"], d["W_off"], d["b_off"], d["W_mask"], d["b_mask"])
    exp = d["exp"]
    err = np.abs(got - exp).max()
    print("absmax err:", err, "rel:", err / np.abs(exp).max())


# revision 5
# speedup vs baseline: 6.5495x; 2.1756x over previous
"""DCNv3 forward on 8 trn2 NeuronCores.

Strategy (data-parallel over (batch, H-half) -> 8 shards):
  - host: pre-transpose per-shard input into the layouts the device wants
    (zero-padded bf16 pixel slab with (cc,g)-interleaved channels for
    sampling; CHW bf16 tile for the offset/mask matmuls)
  - device per core, per 4-row tile:
      PE matmul  : offsets (72) + mask logits (36) for 512 pixels (bf16)
      ACT        : exp, hat weights relu(1 - |u - t|) for a COMMON absolute
                   tap window (UxV cells shared by every (g,p))
      DVE (bf16) : per-cell coefficients A[q,g,u,v] = sum_p m_p*hy_p(u)*hx_p(v)
                   via one big outer-product mult + one reduce over p;
                   apply = per used cell one 512-elem mult (2x_1p mode:
                   all inner dims stride-1 thanks to the channel interleave),
                   then a binary tree of contiguous adds over cells.
  - bilinear gather is gather-free: integer parts of all sampling offsets are
    bounded, so sampling = hat-weighted fixed shifts of the input; every fixed
    shift is an access-pattern offset into an SBUF slab (V column-shifted
    copies of the row slab).
"""

import numpy as np
import sys

sys.path.insert(0, "/opt/trn_rl_repo")

import concourse.bass as bass
import concourse.bacc as bacc
import concourse.mybir as mybir
import concourse.tile as tile
from concourse.bass_utils import run_bass_kernel_spmd

B, C, H, W = 4, 128, 128, 128
G, P, gc = 4, 9, 32
N_CORES = 8
HS = H // 2          # rows per core (b, half)
RT = 4               # output rows per device tile
NTILES = HS // RT    # 16

f32 = mybir.dt.float32
bf16 = mybir.dt.float16

_KS = np.array([-1.0, 0.0, 1.0], np.float32)
KX = np.repeat(_KS, 3)   # x-major flatten (matches torch meshgrid in ref)
KY = np.tile(_KS, 3)


def _geometry(inp, W_off, b_off):
    """Global tap window + used-cell mask from the actual offset field."""
    xhw = inp.reshape(B, H, W, C)
    off = (xhw.reshape(-1, C) @ W_off + b_off).reshape(-1, G, P, 2)
    rx = off[..., 0] + KX          # offset (x) relative to wo+1  (padded coords)
    ry = off[..., 1] + KY
    Bx = np.floor(rx.min(axis=0)).astype(np.int64)
    By = np.floor(ry.min(axis=0)).astype(np.int64)
    spx = np.floor(rx.max(axis=0)).astype(np.int64) + 2 - Bx
    spy = np.floor(ry.max(axis=0)).astype(np.int64) + 2 - By
    g = _Geom()
    g.DX0 = int(Bx.min())
    g.DY0 = int(By.min())
    g.V = int((Bx + spx).max()) - g.DX0
    g.U = int((By + spy).max()) - g.DY0
    used = np.zeros((g.U, g.V), bool)
    for gg in range(G):
        for p in range(P):
            u0 = By[gg, p] - g.DY0
            v0 = Bx[gg, p] - g.DX0
            used[u0:u0 + spy[gg, p], v0:v0 + spx[gg, p]] = True
    g.cells = [(u, v) for u in range(g.U) for v in range(g.V) if used[u, v]]
    return g


class _Geom:
    pass


def _build(g: "_Geom"):
    nc = bacc.Bacc("TRN2", target_bir_lowering=False, debug=False,
                   num_devices=N_CORES)

    U, V, NR = g.U, g.V, g.NR
    NCELL = len(g.cells)
    NTAP = max(U, V)

    xslab_t = nc.dram_tensor("xslab", [g.NROW * g.NCOL * C], bf16, kind="ExternalInput")
    xchw_t = nc.dram_tensor("xchw", [C, HS * W], bf16, kind="ExternalInput")
    wcat_t = nc.dram_tensor("wcat", [C, 108], bf16, kind="ExternalInput")
    addc_t = nc.dram_tensor("addc", [C, 108], f32, kind="ExternalInput")
    cvals_t = nc.dram_tensor("cvals", [C, NTAP + 1], f32, kind="ExternalInput")
    out_t = nc.dram_tensor("out", [HS * W * C], bf16, kind="ExternalOutput")

    mult, add = mybir.AluOpType.mult, mybir.AluOpType.add
    AF = mybir.ActivationFunctionType

    def vap(v, off, dims):
        return bass.AP(tensor=v.tensor, offset=v.offset + off, ap=[v.ap[0]] + dims)

    with tile.TileContext(nc) as tc:
        with (
            nc.allow_low_precision(reason="bf16 cell sums, fp32 positions"),
            tc.tile_pool(name="const", bufs=1) as cpool,
            tc.tile_pool(name="xs", bufs=2) as xspool,
            tc.tile_pool(name="work", bufs=2) as wpool,
            tc.tile_pool(name="big", bufs=1) as bpool,
            tc.tile_pool(name="psum", bufs=2, space="PSUM") as pspool,
        ):
            wcat0 = cpool.tile([C, 108], bf16)
            wcat = cpool.tile([C, 108], bf16)
            addc = cpool.tile([C, 108], f32)
            cvals = cpool.tile([C, NTAP + 1], f32)
            nc.sync.dma_start(wcat0[:], wcat_t.ap())
            nc.sync.dma_start(addc[:], addc_t.ap())
            nc.sync.dma_start(cvals[:], cvals_t.ap())
            # matmul operands come via ACT copies: the Matmult HW struct has a
            # single sync-wait slot, so all its deps must arrive on one sem
            nc.scalar.copy(wcat[:], wcat0[:])

            for t in range(NTILES):
                # ---- loads -------------------------------------------------
                xs = xspool.tile([C, V * NR * C], bf16, name="xs")
                for v in range(V):
                    src = bass.AP(
                        tensor=xslab_t,
                        offset=(RT * t * g.NCOL + g.C0 + v) * C,
                        ap=[[C, W], [g.NCOL * C, NR], [1, C]])
                    nc.sync.dma_start(
                        vap(xs[:], v * NR * C, [[C, NR], [1, C]]), src)

                xc0 = wpool.tile([C, RT * W], bf16, name="xc0")
                nc.sync.dma_start(
                    xc0[:], bass.AP(tensor=xchw_t, offset=RT * t * W,
                                    ap=[[HS * W, C], [1, RT * W]]))
                xc = wpool.tile([C, RT * W], bf16, name="xc")
                nc.scalar.copy(xc[:], xc0[:])

                # ---- offsets / logits (PE), directly in q-on-partitions ----
                rawq = wpool.tile([C, RT * 108], f32, name="rawq")
                for k in range(RT):
                    praw = pspool.tile([C, 108], f32, name="praw")
                    nc.tensor.matmul(praw[:], xc[:, k * W:(k + 1) * W],
                                     wcat[:], start=True, stop=True)
                    nc.scalar.copy(vap(rawq[:], k * 108, [[1, 108]]), praw[:])

                # rawq[:, r*108 + k] : k 0..35 y-offs, 36..71 x-offs, 72..107 logits
                nc.vector.tensor_tensor(
                    vap(rawq[:], 0, [[108, RT], [1, 108]]),
                    vap(rawq[:], 0, [[108, RT], [1, 108]]),
                    vap(addc[:], 0, [[0, RT], [1, 108]]), add)

                # ---- softmax (unnormalized exp -> normalize) ---------------
                el = wpool.tile([C, RT * 36], bf16, name="el")
                nc.scalar.activation(
                    vap(el[:], 0, [[36, RT], [1, 36]]),
                    vap(rawq[:], 72, [[108, RT], [1, 36]]), AF.Exp)
                den = wpool.tile([C, RT * G], f32, name="den")
                nc.vector.tensor_reduce(
                    vap(den[:], 0, [[G, RT], [1, G]]),
                    vap(el[:], 0, [[36, RT], [9, G], [1, P]]),
                    mybir.AxisListType.X, add)
                denr = wpool.tile([C, RT * G], f32, name="denr")
                nc.vector.reciprocal(denr[:], den[:])
                nc.vector.tensor_tensor(
                    vap(el[:], 0, [[36, RT], [9, G], [1, P]]),
                    vap(el[:], 0, [[36, RT], [9, G], [1, P]]),
                    vap(denr[:], 0, [[G, RT], [1, G], [0, P]]), mult)

                # ---- hat weights at absolute taps 0..NTAP-1 (y and x) ------
                # hyx[wo, t*RT*72 + r*72 + (0..35 y | 36..71 x)]
                hyx = wpool.tile([C, NTAP * RT * 72], bf16, name="hyx")
                habs = wpool.tile([C, NTAP * RT * 72], f32, name="habs")
                for tt in range(NTAP):
                    nc.scalar.activation(
                        vap(habs[:], tt * RT * 72, [[72, RT], [1, 72]]),
                        vap(rawq[:], 0, [[108, RT], [1, 72]]),
                        AF.Abs, bias=cvals[:, tt:tt + 1])
                    nc.scalar.activation(
                        vap(hyx[:], tt * RT * 72, [[72, RT], [1, 72]]),
                        vap(habs[:], tt * RT * 72, [[72, RT], [1, 72]]),
                        AF.Relu, bias=cvals[:, NTAP:NTAP + 1], scale=-1.0)

                # compact hx: hxc[wo, v*RT*36 + r*36 + gp]
                hxc = wpool.tile([C, V * RT * 36], bf16, name="hxc")
                nc.scalar.copy(
                    vap(hxc[:], 0, [[RT * 36, V], [36, RT], [1, 36]]),
                    vap(hyx[:], 36, [[RT * 72, V], [72, RT], [1, 36]]))

                # ---- mh[u] = m * hy_u  (one op over (u, r, gp)) ------------
                mh = wpool.tile([C, U * RT * 36], bf16, name="mh")
                nc.vector.tensor_tensor(
                    vap(mh[:], 0, [[RT * 36, U], [36, RT], [1, 36]]),
                    vap(el[:], 0, [[0, U], [36, RT], [1, 36]]),
                    vap(hyx[:], 0, [[RT * 72, U], [72, RT], [1, 36]]), mult)

                # ---- A[q,g,u,v] = sum_p mh_u,p * hx_v,p --------------------
                # prod[wo, (u*V+v)*144 + r*36 + gp]
                prod = bpool.tile([C, U * V * RT * 36], bf16, name="prod")
                nc.vector.tensor_tensor(
                    vap(prod[:], 0, [[V * RT * 36, U], [RT * 36, V], [1, RT * 36]]),
                    vap(mh[:], 0, [[RT * 36, U], [0, V], [1, RT * 36]]),
                    vap(hxc[:], 0, [[0, U], [RT * 36, V], [1, RT * 36]]), mult)
                # A[wo, (u*V+v)*16 + r*4 + g]
                A = wpool.tile([C, U * V * RT * G], bf16, name="A")
                nc.vector.tensor_reduce(
                    vap(A[:], 0, [[RT * G, U * V], [1, RT * G]]),
                    vap(prod[:], 0, [[RT * 36, U * V], [P, RT * G], [1, P]]),
                    mybir.AxisListType.X, add)

                # ---- apply -------------------------------------------------
                # channels are host-interleaved: slab channel index = cc*G + gg
                # tmp[wo, j*512 + r*128 + cc*4 + g]  (contiguous per cell)
                tmp = bpool.tile([C, NCELL * RT * C], bf16, name="tmp")
                for j, (u, v) in enumerate(g.cells):
                    rho0 = 2 + g.DY0 + u
                    nc.vector.tensor_tensor(
                        vap(tmp[:], j * RT * C,
                            [[C, RT], [G, gc], [1, G]]),
                        vap(xs[:], (v * NR + rho0) * C,
                            [[C, RT], [G, gc], [1, G]]),
                        vap(A[:], (u * V + v) * RT * G,
                            [[G, RT], [0, gc], [1, G]]), mult)

                # binary tree of contiguous adds over cells -> tmp[:, 0:512]
                n = NCELL
                while n > 1:
                    half = n // 2
                    nc.vector.tensor_tensor(
                        vap(tmp[:], 0, [[1, half * RT * C]]),
                        vap(tmp[:], 0, [[1, half * RT * C]]),
                        vap(tmp[:], half * RT * C, [[1, half * RT * C]]), add)
                    n = half

                nc.sync.dma_start(
                    bass.AP(tensor=out_t, offset=RT * t * W * C,
                            ap=[[C, W], [W * C, RT], [1, C]]),
                    vap(tmp[:], 0, [[C, RT], [1, C]]))

    nc.compile()
    return nc


def _host_prep(inp, W_off, b_off, W_mask, b_mask, g):
    xhw = inp.reshape(B, H, W, C)
    NTAP = max(g.U, g.V)

    wcat = np.empty((C, 108), np.float32)
    addc_row = np.empty(108, np.float32)
    for gg in range(G):
        for p in range(P):
            gp = gg * P + p
            wcat[:, gp] = W_off[:, 2 * gp + 1]           # y
            wcat[:, 36 + gp] = W_off[:, 2 * gp]          # x
            wcat[:, 72 + gp] = W_mask[:, gp]
            addc_row[gp] = b_off[2 * gp + 1] + (KY[p] - g.DY0)
            addc_row[36 + gp] = b_off[2 * gp] + (KX[p] - g.DX0)
            addc_row[72 + gp] = b_mask[gp]
    addc = np.tile(addc_row[None, :], (C, 1))
    cvals = np.zeros((C, NTAP + 1), np.float32)
    for i in range(NTAP):
        cvals[:, i] = -float(i)
    cvals[:, NTAP] = 1.0

    import ml_dtypes
    bfl = np.float16
    # channel interleave: new channel index cc*G + gg  <- old gg*gc + cc
    perm = np.arange(C).reshape(G, gc).T.reshape(-1)   # perm[new] = old

    in_maps = []
    for core in range(N_CORES):
        b, half = divmod(core, 2)
        h0 = HS * half
        # slab rows: padded rows [h0-1, h0-1+NROW) ; cols: padded [-2, NCOL-2)
        xslab = np.zeros((g.NROW, g.NCOL, C), bfl)
        for lr in range(g.NROW):
            orig = lr + h0 - 2
            if 0 <= orig < H:
                xslab[lr, 3:3 + W, :] = xhw[b, orig][:, perm].astype(bfl)
        xchw = np.ascontiguousarray(
            xhw[b, h0:h0 + HS].reshape(HS * W, C).T).astype(bfl)
        in_maps.append({
            "xslab": xslab.reshape(-1),
            "xchw": xchw,
            "wcat": wcat.astype(bfl),
            "addc": addc,
            "cvals": cvals,
        })
    return in_maps


def _make_geom(inp, W_off, b_off):
    g = _geometry(inp, W_off, b_off)
    # slab row for output row r (in tile), tap u: rho = r + 2 + DY0 + u
    rmin = 2 + g.DY0
    assert rmin >= 0
    g.NR = RT - 1 + 2 + g.DY0 + g.U - 1 + 1   # max rho + 1
    g.NROW = RT * (NTILES - 1) + g.NR     # slab rows per core
    # slab col for (wo, v): wo + v + (3 + DX0)
    g.C0 = 3 + g.DX0                      # col offset baked into slab layout
    assert g.C0 >= 0
    g.NCOL = W + g.V - 1 + g.C0 + 1
    return g


def _run(inp, W_off, b_off, W_mask, b_mask, **spmd_kwargs):
    inp = np.ascontiguousarray(inp, np.float32)
    g = _make_geom(inp, np.asarray(W_off, np.float32), np.asarray(b_off, np.float32))
    nc = _build(g)
    in_maps = _host_prep(inp, np.asarray(W_off, np.float32),
                         np.asarray(b_off, np.float32),
                         np.asarray(W_mask, np.float32),
                         np.asarray(b_mask, np.float32), g)
    res = run_bass_kernel_spmd(nc, in_maps, core_ids=list(range(N_CORES)),
                               **spmd_kwargs)
    # inverse channel interleave: out channel position cc*G + gg
    perm = np.arange(C).reshape(G, gc).T.reshape(-1)
    inv = np.empty(C, np.int64)
    inv[perm] = np.arange(C)
    out = np.empty((B, H, W, C), np.float32)
    for core in range(N_CORES):
        b, half = divmod(core, 2)
        o = res.results[core]["out"].astype(np.float32).reshape(HS, W, C)
        out[b, HS * half:HS * (half + 1)] = o[:, :, inv]
    return out.reshape(B, C, H, W), res


def kernel(inp, W_off, b_off, W_mask, b_mask):
    out, _ = _run(inp, W_off, b_off, W_mask, b_mask)
    return out


if __name__ == "__main__":
    d = np.load("/root/problem/ref_cache.npz")
    got = kernel(d["inp"], d["W_off"], d["b_off"], d["W_mask"], d["b_mask"])
    exp = d["exp"]
    err = np.abs(got - exp).max()
    print("absmax err:", err, "rel:", err / np.abs(exp).max())
